# revision 36
# baseline (speedup 1.0000x reference)
"""Graph Wavelet Neural Network forward pass on 8 Trainium2 NeuronCores.

Computation: out = wavelets @ diag(filt) @ wavelets_inv @ features @ W
  N=8192, C_IN=256, C_OUT=128.

Strategy (memory regime: streaming the two [8192,8192] matrices dominates):
  - Core j owns row-block jb of wavelets_inv (-> right rows jb) and
    column-block jb of wavelets (-> full-shape partial of out; host sums
    the 8 partials). No device collectives.
  - Operands are pre-transposed/pre-blocked on the host so the contraction
    index lands on SBUF partitions and EVERY device DMA is one fully
    contiguous block:
      ft     = features.T                  [256, 8192]   (replicated)
      winv_t = (filt * wavelets_inv)[jb].T  [8192, 1024]  (per-core)
      wav_b  = wavelets[:, jb].T chunk-major [8*1024, 1024] (per-core),
               row ncch*1024 + m holds wav_t[m, ncch*1024 : ...]
    filt is folded into wavelets_inv rows on the host (free O(N^2)).
  - Big streams are bf16: halves HBM traffic (the roofline) and runs the
    PE at 1 cycle/row. PSUM accumulation stays fp32. Output partials are
    bf16 (their host fp64 sum adds ~1e-4 relative error) and leave in a
    chunk-major [8*128, 1024] layout so writes are contiguous too.
  - Device pipeline (core j):
      T    = features @ W              PE form A, T k-tiles in SBUF
      SR^T = sum_k T[k].T @ winv_t[k]  [128, 1024] psum accumulation
      SR   = PE-transpose(SR^T)        8 tiles [128m, 128c]
      o^T  = sum_m SR[m].T @ wav[m, nch]  per 1024-wide n-chunk
    Stage A groups interleave with stage B consumers in PE program order.
    Bulk DMAs are 1MB contiguous, alternating the two HWDGE rings.
"""

import os

import numpy as np

import concourse.bass as bass
import concourse.mybir as mybir
import concourse.tile as tile
from concourse import bacc
from concourse.bass_utils import run_bass_kernel_spmd

N = 8192
C_IN = 256
C_OUT = 128
M = 8  # cores
B = N // M  # 1024 rows per core
KT = N // 128  # 64 contraction tiles
MT = B // 128  # 8 row tiles per core block
NCH = 1024  # output free-dim chunk
NC = N // NCH  # 8 chunks
F32 = mybir.dt.float32

STREAM = "bf16"  # "bf16" or "f32r" for the big streamed operands

_cache = {}


def _stream_dt():
    return mybir.dt.bfloat16 if STREAM == "bf16" else mybir.dt.float32r


def _stream_np():
    if STREAM == "bf16":
        import ml_dtypes

        return ml_dtypes.bfloat16
    return np.float32


def _build():
    SDT = _stream_dt()
    nc = bacc.Bacc("TRN2", target_bir_lowering=False, debug=False)
    ft = nc.dram_tensor("ft", [C_IN, N], SDT, kind="ExternalInput")
    wm = nc.dram_tensor("wm", [C_IN, C_OUT], SDT, kind="ExternalInput")
    winv_t = nc.dram_tensor("winv_t", [N, B], SDT, kind="ExternalInput")
    wav_b = nc.dram_tensor("wav_b", [NC * B, NCH], SDT, kind="ExternalInput")
    ident_d = nc.dram_tensor("ident", [128, 128], SDT, kind="ExternalInput")
    outp = nc.dram_tensor("outp", [NC * C_OUT, NCH], SDT, kind="ExternalOutput")

    with tile.TileContext(nc) as tc:
        with (
            tc.tile_pool(name="const", bufs=1) as cpool,
            tc.tile_pool(name="stream", bufs=4) as spool,
            tc.tile_pool(name="opool", bufs=3) as opool,
            tc.tile_pool(name="ps_small", bufs=2, space="PSUM") as ps_small,
            tc.tile_pool(name="ps_r", bufs=1, space="PSUM") as ps_r,
            tc.tile_pool(name="ps_o", bufs=2, space="PSUM") as ps_o,
        ):
            # --- constants / small inputs ---
            # identity arrives by DMA: keeps GpSimd entirely out of the
            # kernel (smaller end-of-kernel engine barrier).
            ident = cpool.tile([128, 128], SDT, tag="ident")
            nc.scalar.dma_start(out=ident, in_=ident_d.ap())
            wm_sb = cpool.tile([128, 2 * C_OUT], SDT, tag="wm")
            for k2 in range(2):
                nc.scalar.dma_start(
                    out=wm_sb[:, k2 * C_OUT : (k2 + 1) * C_OUT],
                    in_=wm.ap()[k2 * 128 : (k2 + 1) * 128, :],
                )
            # ft split into quarter-column DMAs on both rings so stage A's
            # first groups unblock early instead of waiting for all 4MB.
            ft_sb = [
                cpool.tile([128, N], SDT, tag=f"ft{k2}", name=f"ft_sb{k2}")
                for k2 in range(2)
            ]
            for q in range(4):
                cols = slice(q * (N // 4), (q + 1) * (N // 4))
                for k2 in range(2):
                    eng = nc.sync if (2 * q + k2) % 2 == 0 else nc.scalar
                    eng.dma_start(
                        out=ft_sb[k2][:, cols],
                        in_=ft.ap()[k2 * 128 : (k2 + 1) * 128, cols],
                    )

            # --- PE warmup: dense dummy matmuls while waiting on ft DMAs ---
            # The HAM clock gate defaults to 1.2 GHz and needs ~3.4us of
            # sustained PE activity to release to 2.4 GHz. The PE is idle
            # during the NEFF preamble + first ft DMAs anyway; warm it.
            ps_w = ps_small.tile([128, 128], F32, tag="psA")
            for _ in range(28):
                nc.tensor.matmul(ps_w, ident, ident, start=True, stop=True)

            # --- stages A+B interleaved in PE program order ---
            # A-group g produces T k-tiles [4g, 4g+4); the 1MB wi DMA that
            # covers exactly those four k-tiles follows immediately.
            t_sb = [
                cpool.tile([128, 4 * 128], SDT, tag=f"T{g}", name=f"t_sb{g}")
                for g in range(KT // 4)
            ]
            ps_sr = ps_r.tile([128, B], F32, tag="psR")
            for g in range(KT // 4):
                ps = ps_small.tile([128, 512], F32, tag="psA")
                for i in range(4):
                    n_tile = g * 4 + i
                    for k2 in range(2):
                        nc.tensor.matmul(
                            ps[:, i * 128 : (i + 1) * 128],
                            ft_sb[k2][:, n_tile * 128 : (n_tile + 1) * 128],
                            wm_sb[:, k2 * C_OUT : (k2 + 1) * C_OUT],
                            start=(k2 == 0),
                            stop=(k2 == 1),
                        )
                nc.vector.tensor_copy(t_sb[g], ps)
                wi = spool.tile([128, 4 * B], SDT, tag="wi", bufs=8)
                src = winv_t.ap()[g * 512 : (g + 1) * 512, :].rearrange(
                    "(a p) f -> p a f", a=4
                )
                eng = nc.sync if g % 2 == 0 else nc.scalar
                eng.dma_start(out=wi.rearrange("p (a f) -> p a f", a=4), in_=src)
                for a in range(4):
                    k = 4 * g + a
                    lhs = t_sb[g][:, a * 128 : (a + 1) * 128]
                    for h in range(2):
                        nc.tensor.matmul(
                            ps_sr[:, h * 512 : (h + 1) * 512],
                            lhs,
                            wi[:, a * B + h * 512 : a * B + (h + 1) * 512],
                            start=(k == 0),
                            stop=(k == KT - 1),
                        )

            srT = cpool.tile([128, B], SDT, tag="srT")
            nc.vector.tensor_copy(srT, ps_sr)

            # --- stage C: SR tiles = transpose(SR^T) ---
            sr_sb = [
                cpool.tile([128, 128], SDT, tag=f"sr{mt}", name=f"sr_sb{mt}")
                for mt in range(MT)
            ]
            for mt in range(MT):
                pst = ps_small.tile([128, 128], SDT, tag="psA")
                nc.tensor.transpose(pst, srT[:, mt * 128 : (mt + 1) * 128], ident)
                nc.vector.tensor_copy(sr_sb[mt], pst)

            # --- stage D: out^T partial chunks; all DMAs 1MB contiguous ---
            for ncch in range(NC):
                ps_out = ps_o.tile([128, NCH], F32, tag="psO")
                for mg in range(2):
                    wv = spool.tile([128, 4 * NCH], SDT, tag="wv", bufs=8)
                    src = wav_b.ap()[
                        ncch * B + mg * 512 : ncch * B + (mg + 1) * 512, :
                    ].rearrange("(a p) f -> p a f", a=4)
                    eng = nc.sync if (ncch * 2 + mg) % 2 == 0 else nc.scalar
                    eng.dma_start(out=wv.rearrange("p (a f) -> p a f", a=4), in_=src)
                    for a in range(4):
                        mt = 4 * mg + a
                        for h in range(NCH // 512):
                            nc.tensor.matmul(
                                ps_out[:, h * 512 : (h + 1) * 512],
                                sr_sb[mt],
                                wv[:, a * NCH + h * 512 : a * NCH + (h + 1) * 512],
                                start=(mt == 0),
                                stop=(mt == MT - 1),
                            )
                ot = opool.tile([128, NCH], SDT, tag="ot")
                nc.vector.tensor_copy(ot, ps_out)
                # SWDGE queue: keeps result writes out of the two HWDGE
                # rings, whose FIFO order would stall pending input DMAs
                # behind a CAST-gated output DMA (head-of-line blocking).
                nc.gpsimd.dma_start(
                    out=outp.ap()[ncch * C_OUT : (ncch + 1) * C_OUT, :], in_=ot
                )
    nc.compile()
    return nc


def make_in_maps(features, wavelets, wavelets_inv, weight_matrix, filt):
    sdt = _stream_np()
    features = np.ascontiguousarray(features, dtype=np.float32)
    wavelets = np.ascontiguousarray(wavelets, dtype=np.float32)
    wavelets_inv = np.ascontiguousarray(wavelets_inv, dtype=np.float32)
    weight_matrix = np.ascontiguousarray(weight_matrix, dtype=np.float32)
    filt = np.ascontiguousarray(filt, dtype=np.float32)

    ft = np.ascontiguousarray(features.T).astype(sdt)
    wm = weight_matrix.astype(sdt)
    in_maps = []
    for j in range(M):
        jb = slice(j * B, (j + 1) * B)
        winv_t = np.ascontiguousarray((wavelets_inv[jb, :] * filt[jb, None]).T).astype(sdt)
        # chunk-major blocking of wavelets[:, jb].T: row ncch*B + m
        wav_t = wavelets[:, jb].T  # [B, N]
        wav_b = np.ascontiguousarray(
            wav_t.reshape(B, NC, NCH).transpose(1, 0, 2).reshape(NC * B, NCH)
        ).astype(sdt)
        in_maps.append(
            {"ft": ft, "wm": wm, "winv_t": winv_t, "wav_b": wav_b,
             "ident": np.eye(128, dtype=np.float32).astype(sdt)}
        )
    return in_maps


def combine_outputs(results):
    acc = results[0]["outp"].astype(np.float64)
    for j in range(1, M):
        acc += results[j]["outp"]
    # outp rows are [ncch][c]: row ncch*C_OUT + c holds out^T[c, ncch*NCH:...]
    out_t = acc.reshape(NC, C_OUT, NCH).transpose(1, 0, 2).reshape(C_OUT, N)
    return np.ascontiguousarray(out_t.T.astype(np.float32))


def kernel(features, wavelets, wavelets_inv, weight_matrix, filt):
    os.environ.setdefault("BASS_NEVER_TRACE", "1")
    if "nc" not in _cache:
        _cache["nc"] = _build()
    nc = _cache["nc"]
    in_maps = make_in_maps(features, wavelets, wavelets_inv, weight_matrix, filt)
    res = run_bass_kernel_spmd(nc, in_maps, core_ids=list(range(M)))
    return combine_outputs(res.results)
